# revision 32
# baseline (speedup 1.0000x reference)
"""MoE (cosine-routed, top-k, 2-layer GELU FFN) on 8 Trainium2 NeuronCores.

Strategy (expert-parallel, per the sharding hint):
  - Host computes the (tiny) routing: cosine scores -> softmax -> top-k ->
    renormalized gate weights. ~34 MFLOP, negligible vs the 34 GFLOP FFN.
  - Tokens are dispatched by top-k expert id: core e receives the tokens
    routed to expert e (padded to capacity C), plus expert e's W1/b1/W2.
  - Each core runs the 2-layer FFN in bf16 (fp32 PSUM accumulation); the
    per-token gate weight is folded into h between the two GEMMs.
  - Host scatter-adds the (<= top_k) expert contributions per token and
    adds the gate-weighted b2 (exact, since b2 is per-expert).

Device pipeline per core (P = 128 partitions):
  GEMM1: hT[f, t] = sum_d W1[d, f] * xT[d, t]   (W1 tiles stationary)
         -> Gelu(. + b1) on ScalarE -> x gate (bf16) on VectorE
  GEMM2: yT[d, t] = sum_f W2[f, d] * hT[f, t]   (W2 tiles stationary)
         -> plain PSUM->SBUF copy, bf16 out, DMA to HBM

Perf notes (trace-driven; ~76.5us vs the 84.3us predecessor):
  - The token dim C (544 here) exceeds one PSUM bank (512 fp32), so each
    (f, d) stationary tile serves 2 chunked matmuls; equal chunks
    (272+272) cost ~232ns/pair (N/2.4GHz + ~2.5ns NX floor per matmul).
    LDWEIGHTS fully hides under the PE's reorder window.
  - Input DMA runs at the HBM roofline (~360 GB/s aggregate; pending
    DMAs share it ~round-robin per packet). Instead of waiting ~4us for
    all of x + w1_f0 to land, GEMM1 starts in a d-outer "head phase":
    the first PHF f-blocks accumulate across all 8 PSUM banks, consuming
    one x d-block + one W1 d-slice (~270KB) per ~0.93us - a rate the
    stream sustains - so real work starts when the first ~270KB lands.
  - A PE warm-up (dummy matmuls, no DMA deps) bridges the framework
    preamble (~7.5us) to first-data (~11.3us): the HAM clock gate
    releases (1.2 -> 2.4 GHz) only after ~3.5-4.5us of sustained PE
    busy, and any idle gap resets that window. Mid-stream stalls must
    stay well under 3.4us or HAM re-throttles (costs double).
  - Tail: GEMM2's epilogue is a copy (gate already applied), so the last
    d-block's two chunks copy in parallel on VectorE + ScalarE (distinct
    PSUM banks) and store via both HWDGE rings (Sync + Scalar engines).
  - Output is bf16 (error contribution ~0.2-0.4%; total rel err 4.5e-3,
    well inside the 2e-2 budget).
"""

import numpy as np
import ml_dtypes

P = 128
D_MODEL = 1024
D_FF = 2048
N_EXPERTS = 8
N_CORES = 8
N_WARMUP_MM = 9       # 512-col dummy matmuls @ cold 1.2GHz ~= 3.8us
                      # (bridges preamble-end ~7.5us to first-data ~11.3us;
                      # an idle gap there would reset the HAM busy window,
                      # delaying full clock by another ~4.3us)
WARMUP_COLS = 512

_BF16 = ml_dtypes.bfloat16

_cache: dict = {}
last_results = None  # BassKernelResults of the most recent run (for profiling)


def _chunks(C):
    """Split C into equal-ish 16-aligned chunks of <=512 (PSUM bank)."""
    n = -(-C // 512)
    base = -(-C // (16 * n)) * 16
    out = []
    c0 = 0
    while c0 < C:
        cw = min(base, C - c0)
        out.append((c0, cw))
        c0 += cw
    return out


def _phf(C):
    """How many leading f-blocks run d-outer at the stream head (each
    needs len(_chunks) open PSUM banks; 8 banks total)."""
    return max(1, 8 // len(_chunks(C)))


def _w1_off(f, d, PHF, ND):
    """Column offset of the (f, d) 128x128 W1 tile in the phase-ordered
    w1 DRAM/SBUF layout: the first PHF f-blocks are packed d-major
    (d-slice by d-slice), the rest f-major (f-block by f-block)."""
    if f < PHF:
        return (d * PHF + f) * P
    return PHF * ND * P + (f - PHF) * ND * P + d * P


def _build(C):
    """Build + compile the SPMD FFN kernel for capacity C (multiple of 32)."""
    import concourse.mybir as mybir
    from concourse import bacc
    from concourse.tile import TileContext

    D, F = D_MODEL, D_FF
    ND, NF = D // P, F // P

    nc = bacc.Bacc("TRN2", target_bir_lowering=False, debug=False,
                   enable_partition_id=False)

    # Host-pre-arranged layouts (see kernel() for the packing):
    #   xT:  [P, ND*C]    column d*C + t = x[token t, d*P + part]
    #   w1:  [P, NF*ND*P] phase-ordered (see _w1_off)
    #   w2:  [P, NF*D]    column f*D + j = W2[f*P + part, j]
    xT_d = nc.dram_tensor("xT", [P, ND * C], mybir.dt.bfloat16, kind="ExternalInput")
    w1_d = nc.dram_tensor("w1", [P, NF * ND * P], mybir.dt.bfloat16,
                          kind="ExternalInput")
    w2_d = nc.dram_tensor("w2", [P, NF * D], mybir.dt.bfloat16, kind="ExternalInput")
    meta_d = nc.dram_tensor("meta", [P, NF + ND + C], mybir.dt.float32,
                            kind="ExternalInput")
    out_d = nc.dram_tensor("out", [D, C], mybir.dt.bfloat16, kind="ExternalOutput")

    ck = _chunks(C)
    nck = len(ck)
    PHF = _phf(C)

    def w1off(f, d):
        return _w1_off(f, d, PHF, ND)

    with TileContext(nc) as tc:
        with (
            tc.tile_pool(name="weights", bufs=1) as wp,
            tc.tile_pool(name="acts", bufs=1) as ap,
            tc.tile_pool(name="outs", bufs=4) as op,
            tc.tile_pool(name="psum", bufs=1, space="PSUM") as pp,
        ):
            # 8 shared PSUM slots, addressed as slot j -> tiles per chunk
            def pslot(j, name):
                return [pp.tile([P, cw], mybir.dt.float32,
                                tag=f"S{j * nck + ci}", name=f"{name}_{ci}")
                        for ci, (c0, cw) in enumerate(ck)]

            # --- PE warm-up (see module docstring)
            dummy = ap.tile([P, WARMUP_COLS], mybir.dt.bfloat16, tag="dummy")
            nc.vector.memset(dummy[:], 0.0)
            wps = pp.tile([P, WARMUP_COLS], mybir.dt.float32, tag="S0",
                          name="warm_ps")
            for _ in range(N_WARMUP_MM):
                nc.tensor.matmul(wps[:], dummy[:, :P], dummy[:], start=True,
                                 stop=True)

            xt = ap.tile([P, ND * C], mybir.dt.bfloat16, tag="xt")
            w1t = wp.tile([P, NF * ND * P], mybir.dt.bfloat16, tag="w1")
            w2t = wp.tile([P, NF * D], mybir.dt.bfloat16, tag="w2")
            mt = wp.tile([P, NF + ND + C], mybir.dt.float32, tag="meta")
            b1t = mt[:, 0:NF]
            gt = mt[:, NF + ND : NF + ND + C]
            ht = ap.tile([P, NF * C], mybir.dt.bfloat16, tag="ht")
            # gates as bf16, folded into h after the GELU (VectorE, idle
            # during GEMM1): GEMM2's epilogue is then a plain psum->SBUF
            # copy on ScalarE, keeping VectorE off the final critical
            # path. b2 is applied host-side (g-weighted), exact.
            gtb = ap.tile([P, C], mybir.dt.bfloat16, tag="gtb")

            # --- input DMAs (see module docstring). Sync ring: x pieces,
            # f-major W1 blocks, W2. Scalar ring: phase W1 d-slices, meta.
            W1B = ND * P                   # columns per W1 f-block
            PH = PHF * P                   # phase-region cols per d
            xs = lambda d0, d1: (slice(None), slice(d0 * C, d1 * C))
            w1c = lambda a, b: (slice(None), slice(a, b))

            for dd in range(ND):
                nc.sync.dma_start(out=xt[xs(dd, dd + 1)],
                                  in_=xT_d[xs(dd, dd + 1)])
            r3 = PHF * ND * P              # start of the f-major region
            for f0, f1 in ((PHF, PHF + 1), (PHF + 1, PHF + 2),
                           (PHF + 2, PHF + 4), (PHF + 4, PHF + 8),
                           (PHF + 8, NF)):
                if f0 >= NF:
                    break
                f1 = min(f1, NF)
                a = r3 + (f0 - PHF) * W1B
                b = r3 + (f1 - PHF) * W1B
                nc.sync.dma_start(out=w1t[w1c(a, b)], in_=w1_d[w1c(a, b)])
            NW2 = 4
            w2step = (NF // NW2) * D
            for i in range(NW2):
                nc.sync.dma_start(out=w2t[:, i * w2step : (i + 1) * w2step],
                                  in_=w2_d[:, i * w2step : (i + 1) * w2step])
            for d0, d1 in ((0, 2), (2, 4), (4, 6), (6, 8)):
                nc.scalar.dma_start(out=w1t[w1c(d0 * PH, d1 * PH)],
                                    in_=w1_d[w1c(d0 * PH, d1 * PH)])
            nc.scalar.dma_start(out=mt[:], in_=meta_d[:])
            # (must come after the meta DMA in program order so Tile sees
            # the writer and sequences the copy behind the DMA)
            nc.vector.tensor_copy(gtb[:], gt)

            # --- GEMM1 head phase: f0..PHF-1 accumulate d-outer across
            # all 8 PSUM banks, consuming x d-block by d-block as it
            # arrives instead of waiting for all of x.
            psA = [pslot(j, f"ph{j}") for j in range(PHF)]
            for d in range(ND):
                for j in range(PHF):
                    lhs = w1t[:, w1off(j, d) : w1off(j, d) + P]
                    for ci, (c0, cw) in enumerate(ck):
                        nc.tensor.matmul(
                            psA[j][ci][:],
                            lhs,
                            xt[:, d * C + c0 : d * C + c0 + cw],
                            start=(d == 0),
                            stop=(d == ND - 1),
                        )
            def gelu_gate(f, ps):
                for ci, (c0, cw) in enumerate(ck):
                    hr = ap.tile([P, cw], mybir.dt.bfloat16, tag=f"hraw{ci}",
                                 name=f"hraw_{f}_{ci}", bufs=2)
                    nc.scalar.activation(
                        hr[:],
                        ps[ci][:],
                        mybir.ActivationFunctionType.Gelu,
                        bias=b1t[:, f : f + 1],
                    )
                    nc.vector.tensor_mul(
                        ht[:, f * C + c0 : f * C + c0 + cw],
                        hr[:],
                        gtb[:, c0 : c0 + cw],
                    )

            for j in range(PHF):
                gelu_gate(j, psA[j])

            # --- GEMM1 steady state: remaining f-blocks, f-outer.
            for f in range(PHF, NF):
                ps = pslot(f % PHF, f"g1_{f}")
                for d in range(ND):
                    lhs = w1t[:, w1off(f, d) : w1off(f, d) + P]
                    for ci, (c0, cw) in enumerate(ck):
                        nc.tensor.matmul(
                            ps[ci][:],
                            lhs,
                            xt[:, d * C + c0 : d * C + c0 + cw],
                            start=(d == 0),
                            stop=(d == ND - 1),
                        )
                gelu_gate(f, ps)

            # --- GEMM2: yT[do*P:(do+1)*P, t] (gate already folded into h,
            # b2 applied host-side, so the epilogue is a plain copy).
            # For the last d-block the per-chunk copies run in PARALLEL on
            # VectorE + ScalarE (different PSUM banks) and the stores
            # issue on both HWDGE rings, minimizing the post-stream tail.
            for do in range(ND):
                last = do == ND - 1
                ps2 = pslot(do % PHF, f"g2_{do}")
                for f in range(NF):
                    lhs = w2t[:, f * D + do * P : f * D + (do + 1) * P]
                    for ci, (c0, cw) in enumerate(ck):
                        nc.tensor.matmul(
                            ps2[ci][:],
                            lhs,
                            ht[:, f * C + c0 : f * C + c0 + cw],
                            start=(f == 0),
                            stop=(f == NF - 1),
                        )
                ot = op.tile([P, C], mybir.dt.bfloat16, tag="ot",
                             name=f"ot_{do}")
                for ci, (c0, cw) in enumerate(ck):
                    osl = ot[:, c0 : c0 + cw]
                    if last and ci % 2 == 0:
                        nc.vector.tensor_copy(osl, ps2[ci][:])
                    else:
                        nc.scalar.copy(osl, ps2[ci][:])
                    if last:
                        eng = nc.scalar if ci % 2 else nc.sync
                        eng.dma_start(
                            out=out_d[do * P : (do + 1) * P, c0 : c0 + cw],
                            in_=osl,
                        )
                if not last:
                    nc.sync.dma_start(
                        out=out_d[do * P : (do + 1) * P, :],
                        in_=ot[:],
                    )

    nc.compile()
    return nc


def _get_kernel(C):
    if C not in _cache:
        _cache[C] = _build(C)
    return _cache[C]


def _run_spmd(nc, in_maps):
    """run_bass_kernel_spmd, robust to a BASS_TRACE env the image can't
    serve (missing antenv.axon_hooks / artifact upload): install a best-
    effort NTFF hook shim, and on a trace-path failure fall back to an
    untraced run."""
    import os
    from concourse.bass_utils import run_bass_kernel_spmd

    try:
        import antenv.axon_hooks  # noqa: F401
    except ImportError:
        import sys
        import types
        hook = None
        try:
            from trn_agent_boot.trn_boot import _ntff_profile_via_ctypes
            hook = _ntff_profile_via_ctypes("/opt/axon/libaxon_pjrt.so")
        except Exception:
            hook = None
        mod = types.ModuleType("antenv.axon_hooks")
        mod.get_axon_ntff_profile_hook = lambda: hook
        try:
            import antenv
            antenv.axon_hooks = mod
            sys.modules["antenv.axon_hooks"] = mod
        except ImportError:
            pass

    core_ids = list(range(N_CORES))
    try:
        return run_bass_kernel_spmd(nc, in_maps, core_ids)
    except Exception:
        if os.environ.get("BASS_NEVER_TRACE") == "1":
            raise
        os.environ["BASS_NEVER_TRACE"] = "1"
        try:
            return run_bass_kernel_spmd(nc, in_maps, core_ids)
        finally:
            del os.environ["BASS_NEVER_TRACE"]


def _pack_w1(w1_e, C):
    """Pack one expert's W1 [D, F] into the phase-ordered [P, NF*ND*P]
    layout matching _w1_off."""
    ND, NF = D_MODEL // P, D_FF // P
    PHF = _phf(C)
    w1 = np.asarray(w1_e, dtype=np.float32).astype(_BF16)
    cur = w1.reshape(ND, P, NF, P)                 # [d, p, f, j]
    reg12 = cur[:, :, :PHF, :].transpose(1, 0, 2, 3).reshape(P, ND * PHF * P)
    reg3 = cur[:, :, PHF:, :].transpose(1, 2, 0, 3).reshape(
        P, (NF - PHF) * ND * P)
    return np.ascontiguousarray(np.concatenate([reg12, reg3], axis=1))


def kernel(x, anchors, temperature, W1, b1, W2, b2, top_k):

    x = np.asarray(x)
    B, S, D = x.shape
    T = B * S
    E = np.asarray(anchors).shape[0]
    k = int(np.asarray(top_k))

    xf = np.ascontiguousarray(x.reshape(T, D), dtype=np.float32)

    # ---- routing on host (part of the dispatch decision) ----
    xn = xf / np.maximum(np.linalg.norm(xf, axis=-1, keepdims=True), 1e-8)
    an = np.asarray(anchors, dtype=np.float32)
    an = an / np.maximum(np.linalg.norm(an, axis=-1, keepdims=True), 1e-8)
    scores = (xn @ an.T) * abs(float(np.asarray(temperature)))
    scores -= scores.max(axis=-1, keepdims=True)
    probs = np.exp(scores)
    probs /= probs.sum(axis=-1, keepdims=True)
    topi = np.argsort(-probs, axis=-1, kind="stable")[:, :k]  # ties -> low idx
    topv = np.take_along_axis(probs, topi, axis=-1)
    gw = topv / (topv.sum(axis=-1, keepdims=True) + 1e-6)

    rows_per_e = []
    gates_per_e = []
    for e in range(E):
        mask = topi == e
        rows = np.nonzero(mask.any(axis=-1))[0]
        g = np.where(mask[rows], gw[rows], 0.0).sum(axis=-1).astype(np.float32)
        rows_per_e.append(rows)
        gates_per_e.append(g)

    max_count = max(len(r) for r in rows_per_e)
    C = max(64, -(-max_count // 32) * 32)
    nc = _get_kernel(C)

    # ---- per-core shards, pre-arranged into SBUF layouts ----
    x_bf = xf.astype(_BF16)
    ND, NF = D_MODEL // P, D_FF // P
    in_maps = []
    for e in range(N_CORES):
        rows = rows_per_e[e]
        n = len(rows)
        xT = np.zeros((P, ND * C), dtype=_BF16)
        # [P, ND, C] view: xT[p, d, t] = x[rows[t], d*P + p]
        xv = xT.reshape(P, ND, C)
        xv[:, :, :n] = x_bf[rows].reshape(n, ND, P).transpose(2, 1, 0)
        w1 = _pack_w1(W1[e], C)
        w2 = np.asarray(W2[e], dtype=np.float32).astype(_BF16)
        w2 = np.ascontiguousarray(
            w2.reshape(NF, P, D_MODEL).transpose(1, 0, 2).reshape(P, NF * D_MODEL))
        meta = np.zeros((P, NF + ND + C), dtype=np.float32)
        meta[:, :NF] = np.asarray(b1[e], dtype=np.float32).reshape(NF, P).T
        meta[:, NF : NF + ND] = np.asarray(b2[e], dtype=np.float32).reshape(ND, P).T
        meta[:, NF + ND : NF + ND + n] = gates_per_e[e][None, :]
        in_maps.append({"xT": xT, "w1": w1, "w2": w2, "meta": meta})

    res = _run_spmd(nc, in_maps)
    global last_results
    last_results = res

    # ---- combine (scatter-add the gated expert outputs; device output
    # is gate-weighted W2@h, so add the gate-weighted b2 here) ----
    out = np.zeros((T, D_MODEL), dtype=np.float32)
    for e in range(N_CORES):
        rows = rows_per_e[e]
        n = len(rows)
        if n:
            out[rows] += res.results[e]["out"][:, :n].T.astype(np.float32)
            b2e = np.asarray(b2[e], dtype=np.float32)
            if b2e.any():
                out[rows] += gates_per_e[e][:, None] * b2e[None, :]
    return out.reshape(B, S, D_MODEL)


# revision 34
# speedup vs baseline: 1.0108x; 1.0108x over previous
"""MoE (cosine-routed, top-k, 2-layer GELU FFN) on 8 Trainium2 NeuronCores.

Strategy (expert-parallel, per the sharding hint):
  - Host computes the (tiny) routing: cosine scores -> softmax -> top-k ->
    renormalized gate weights. ~34 MFLOP, negligible vs the 34 GFLOP FFN.
  - Tokens are dispatched by top-k expert id: core e receives the tokens
    routed to expert e (padded to capacity C), plus expert e's W1/b1/W2.
  - Each core runs the 2-layer FFN in bf16 (fp32 PSUM accumulation); the
    per-token gate weight is folded into h between the two GEMMs.
  - Host scatter-adds the (<= top_k) expert contributions per token and
    adds the gate-weighted b2 (exact, since b2 is per-expert).

Device pipeline per core (P = 128 partitions):
  GEMM1: hT[f, t] = sum_d W1[d, f] * xT[d, t]   (W1 tiles stationary)
         -> Gelu(. + b1) on ScalarE -> x gate (bf16) on VectorE
  GEMM2: yT[d, t] = sum_f W2[f, d] * hT[f, t]   (W2 tiles stationary)
         -> plain PSUM->SBUF copy, bf16 out, DMA to HBM

Perf notes (trace-driven; ~76.5us vs the 84.3us predecessor):
  - The token dim C (544 here) exceeds one PSUM bank (512 fp32), so each
    (f, d) stationary tile serves 2 chunked matmuls; equal chunks
    (272+272) cost ~232ns/pair (N/2.4GHz + ~2.5ns NX floor per matmul).
    LDWEIGHTS fully hides under the PE's reorder window.
  - Input DMA runs at the HBM roofline (~360 GB/s aggregate; pending
    DMAs share it ~round-robin per packet). Instead of waiting ~4us for
    all of x + w1_f0 to land, GEMM1 starts in a d-outer "head phase":
    the first PHF f-blocks accumulate across all 8 PSUM banks, consuming
    one x d-block + one W1 d-slice (~270KB) per ~0.93us - a rate the
    stream sustains - so real work starts when the first ~270KB lands.
  - A PE warm-up (dummy matmuls, no DMA deps) bridges the framework
    preamble (~7.5us) to first-data (~11.3us): the HAM clock gate
    releases (1.2 -> 2.4 GHz) only after ~3.5-4.5us of sustained PE
    busy, and any idle gap resets that window. Mid-stream stalls must
    stay well under 3.4us or HAM re-throttles (costs double).
  - Tail: GEMM2's epilogue is a copy (gate already applied), so the last
    d-block's two chunks copy in parallel on VectorE + ScalarE (distinct
    PSUM banks) and store via both HWDGE rings (Sync + Scalar engines).
  - Output is bf16 (error contribution ~0.2-0.4%; total rel err 4.5e-3,
    well inside the 2e-2 budget).
"""

import numpy as np
import ml_dtypes

P = 128
D_MODEL = 1024
D_FF = 2048
N_EXPERTS = 8
N_CORES = 8
N_WARMUP_MM = 9       # 512-col dummy matmuls @ cold 1.2GHz ~= 3.8us
                      # (bridges preamble-end ~7.5us to first-data ~11.3us;
                      # an idle gap there would reset the HAM busy window,
                      # delaying full clock by another ~4.3us)
WARMUP_COLS = 512

_BF16 = ml_dtypes.bfloat16

_cache: dict = {}
last_results = None  # BassKernelResults of the most recent run (for profiling)


def _chunks(C):
    """Split C into equal-ish 16-aligned chunks of <=512 (PSUM bank)."""
    n = -(-C // 512)
    base = -(-C // (16 * n)) * 16
    out = []
    c0 = 0
    while c0 < C:
        cw = min(base, C - c0)
        out.append((c0, cw))
        c0 += cw
    return out


def _phf(C):
    """How many leading f-blocks run d-outer at the stream head (each
    needs len(_chunks) open PSUM banks; 8 banks total)."""
    return max(1, 8 // len(_chunks(C)))


def _w1_off(f, d, PHF, ND):
    """Column offset of the (f, d) 128x128 W1 tile in the phase-ordered
    w1 DRAM/SBUF layout: the first PHF f-blocks are packed d-major
    (d-slice by d-slice), the rest f-major (f-block by f-block)."""
    if f < PHF:
        return (d * PHF + f) * P
    return PHF * ND * P + (f - PHF) * ND * P + d * P


def _build(C):
    """Build + compile the SPMD FFN kernel for capacity C (multiple of 32)."""
    import concourse.mybir as mybir
    from concourse import bacc
    from concourse.tile import TileContext

    D, F = D_MODEL, D_FF
    ND, NF = D // P, F // P

    nc = bacc.Bacc("TRN2", target_bir_lowering=False, debug=False,
                   enable_partition_id=False)

    # Host-pre-arranged layouts (see kernel() for the packing):
    #   xT:  [P, ND*C]    column d*C + t = x[token t, d*P + part]
    #   w1:  [P, NF*ND*P] phase-ordered (see _w1_off)
    #   w2:  [P, NF*D]    column f*D + j = W2[f*P + part, j]
    xT_d = nc.dram_tensor("xT", [P, ND * C], mybir.dt.bfloat16, kind="ExternalInput")
    w1_d = nc.dram_tensor("w1", [P, NF * ND * P], mybir.dt.bfloat16,
                          kind="ExternalInput")
    w2_d = nc.dram_tensor("w2", [P, NF * D], mybir.dt.bfloat16, kind="ExternalInput")
    meta_d = nc.dram_tensor("meta", [P, NF + ND + C], mybir.dt.float32,
                            kind="ExternalInput")
    out_d = nc.dram_tensor("out", [D, C], mybir.dt.bfloat16, kind="ExternalOutput")

    ck = _chunks(C)
    nck = len(ck)
    PHF = _phf(C)

    def w1off(f, d):
        return _w1_off(f, d, PHF, ND)

    with TileContext(nc) as tc:
        with (
            tc.tile_pool(name="weights", bufs=1) as wp,
            tc.tile_pool(name="acts", bufs=1) as ap,
            tc.tile_pool(name="outs", bufs=4) as op,
            tc.tile_pool(name="psum", bufs=1, space="PSUM") as pp,
        ):
            # 8 shared PSUM slots, addressed as slot j -> tiles per chunk
            def pslot(j, name):
                return [pp.tile([P, cw], mybir.dt.float32,
                                tag=f"S{j * nck + ci}", name=f"{name}_{ci}")
                        for ci, (c0, cw) in enumerate(ck)]

            # --- PE warm-up (see module docstring)
            dummy = ap.tile([P, WARMUP_COLS], mybir.dt.bfloat16, tag="dummy")
            nc.vector.memset(dummy[:], 0.0)
            wps = pp.tile([P, WARMUP_COLS], mybir.dt.float32, tag="S0",
                          name="warm_ps")
            for _ in range(N_WARMUP_MM):
                nc.tensor.matmul(wps[:], dummy[:, :P], dummy[:], start=True,
                                 stop=True)

            xt = ap.tile([P, ND * C], mybir.dt.bfloat16, tag="xt")
            w1t = wp.tile([P, NF * ND * P], mybir.dt.bfloat16, tag="w1")
            w2t = wp.tile([P, NF * D], mybir.dt.bfloat16, tag="w2")
            mt = wp.tile([P, NF + ND + C], mybir.dt.float32, tag="meta")
            b1t = mt[:, 0:NF]
            gt = mt[:, NF + ND : NF + ND + C]
            ht = ap.tile([P, NF * C], mybir.dt.bfloat16, tag="ht")
            # gates as bf16, folded into h after the GELU (VectorE, idle
            # during GEMM1): GEMM2's epilogue is then a plain psum->SBUF
            # copy on ScalarE, keeping VectorE off the final critical
            # path. b2 is applied host-side (g-weighted), exact.
            gtb = ap.tile([P, C], mybir.dt.bfloat16, tag="gtb")

            # --- input DMAs (see module docstring). Sync ring: x pieces,
            # f-major W1 blocks, W2. Scalar ring: phase W1 d-slices, meta.
            W1B = ND * P                   # columns per W1 f-block
            PH = PHF * P                   # phase-region cols per d
            xs = lambda d0, d1: (slice(None), slice(d0 * C, d1 * C))
            w1c = lambda a, b: (slice(None), slice(a, b))

            for dd in range(ND):
                nc.sync.dma_start(out=xt[xs(dd, dd + 1)],
                                  in_=xT_d[xs(dd, dd + 1)])
            r3 = PHF * ND * P              # start of the f-major region
            for f0, f1 in ((PHF, PHF + 1), (PHF + 1, PHF + 2),
                           (PHF + 2, PHF + 4), (PHF + 4, PHF + 8),
                           (PHF + 8, NF)):
                if f0 >= NF:
                    break
                f1 = min(f1, NF)
                a = r3 + (f0 - PHF) * W1B
                b = r3 + (f1 - PHF) * W1B
                nc.sync.dma_start(out=w1t[w1c(a, b)], in_=w1_d[w1c(a, b)])
            NW2 = 4
            w2step = (NF // NW2) * D
            for i in range(NW2):
                nc.sync.dma_start(out=w2t[:, i * w2step : (i + 1) * w2step],
                                  in_=w2_d[:, i * w2step : (i + 1) * w2step])
            for d0, d1 in ((0, 2), (2, 4), (4, 6), (6, 8)):
                nc.scalar.dma_start(out=w1t[w1c(d0 * PH, d1 * PH)],
                                    in_=w1_d[w1c(d0 * PH, d1 * PH)])
            nc.scalar.dma_start(out=mt[:], in_=meta_d[:])
            # (must come after the meta DMA in program order so Tile sees
            # the writer and sequences the copy behind the DMA)
            nc.vector.tensor_copy(gtb[:], gt)

            # --- GEMM1 head phase: f0..PHF-1 accumulate d-outer across
            # all 8 PSUM banks, consuming x d-block by d-block as it
            # arrives instead of waiting for all of x.
            psA = [pslot(j, f"ph{j}") for j in range(PHF)]
            for d in range(ND):
                for j in range(PHF):
                    lhs = w1t[:, w1off(j, d) : w1off(j, d) + P]
                    for ci, (c0, cw) in enumerate(ck):
                        nc.tensor.matmul(
                            psA[j][ci][:],
                            lhs,
                            xt[:, d * C + c0 : d * C + c0 + cw],
                            start=(d == 0),
                            stop=(d == ND - 1),
                        )
            def gelu_gate(f, ps):
                for ci, (c0, cw) in enumerate(ck):
                    hr = ap.tile([P, cw], mybir.dt.bfloat16, tag=f"hraw{ci}",
                                 name=f"hraw_{f}_{ci}", bufs=2)
                    nc.scalar.activation(
                        hr[:],
                        ps[ci][:],
                        mybir.ActivationFunctionType.Gelu,
                        bias=b1t[:, f : f + 1],
                    )
                    nc.vector.tensor_mul(
                        ht[:, f * C + c0 : f * C + c0 + cw],
                        hr[:],
                        gtb[:, c0 : c0 + cw],
                    )

            for j in range(PHF):
                gelu_gate(j, psA[j])

            # --- GEMM1 steady state: remaining f-blocks, f-outer.
            for f in range(PHF, NF):
                ps = pslot(f % PHF, f"g1_{f}")
                for d in range(ND):
                    lhs = w1t[:, w1off(f, d) : w1off(f, d) + P]
                    for ci, (c0, cw) in enumerate(ck):
                        nc.tensor.matmul(
                            ps[ci][:],
                            lhs,
                            xt[:, d * C + c0 : d * C + c0 + cw],
                            start=(d == 0),
                            stop=(d == ND - 1),
                        )
                gelu_gate(f, ps)

            # --- GEMM2: yT[do*P:(do+1)*P, t] (gate already folded into h,
            # b2 applied host-side, so the epilogue is a plain copy).
            # For the last d-block the per-chunk copies run in PARALLEL on
            # VectorE + ScalarE (different PSUM banks) and the stores
            # issue on both HWDGE rings, minimizing the post-stream tail.
            for do in range(ND):
                last = do == ND - 1
                ps2 = pslot(do % PHF, f"g2_{do}")
                for f in range(NF):
                    lhs = w2t[:, f * D + do * P : f * D + (do + 1) * P]
                    for ci, (c0, cw) in enumerate(ck):
                        nc.tensor.matmul(
                            ps2[ci][:],
                            lhs,
                            ht[:, f * C + c0 : f * C + c0 + cw],
                            start=(f == 0),
                            stop=(f == NF - 1),
                        )
                ot = op.tile([P, C], mybir.dt.bfloat16, tag="ot",
                             name=f"ot_{do}")
                for ci, (c0, cw) in enumerate(ck):
                    osl = ot[:, c0 : c0 + cw]
                    if last and ci % 2 == 0:
                        nc.vector.tensor_copy(osl, ps2[ci][:])
                    else:
                        nc.scalar.copy(osl, ps2[ci][:])
                    if last:
                        eng = nc.scalar if ci % 2 else nc.sync
                        eng.dma_start(
                            out=out_d[do * P : (do + 1) * P, c0 : c0 + cw],
                            in_=osl,
                        )
                if not last:
                    nc.sync.dma_start(
                        out=out_d[do * P : (do + 1) * P, :],
                        in_=ot[:],
                    )

    nc.compile()
    return nc


def _get_kernel(C):
    if C not in _cache:
        _cache[C] = _build(C)
    return _cache[C]


def _run_spmd(nc, in_maps):
    """run_bass_kernel_spmd, robust to a BASS_TRACE env the image can't
    serve (missing antenv.axon_hooks / artifact upload): install a best-
    effort NTFF hook shim, and on a trace-path failure fall back to an
    untraced run."""
    import os
    from concourse.bass_utils import run_bass_kernel_spmd

    try:
        import antenv.axon_hooks  # noqa: F401
    except ImportError:
        import sys
        import types
        hook = None
        try:
            from trn_agent_boot.trn_boot import _ntff_profile_via_ctypes
            hook = _ntff_profile_via_ctypes("/opt/axon/libaxon_pjrt.so")
        except Exception:
            hook = None
        mod = types.ModuleType("antenv.axon_hooks")
        mod.get_axon_ntff_profile_hook = lambda: hook
        try:
            import antenv
            antenv.axon_hooks = mod
            sys.modules["antenv.axon_hooks"] = mod
        except ImportError:
            pass

    core_ids = list(range(N_CORES))
    try:
        return run_bass_kernel_spmd(nc, in_maps, core_ids)
    except Exception:
        if os.environ.get("BASS_NEVER_TRACE") == "1":
            raise
        os.environ["BASS_NEVER_TRACE"] = "1"
        try:
            return run_bass_kernel_spmd(nc, in_maps, core_ids)
        finally:
            del os.environ["BASS_NEVER_TRACE"]


def _pack_w1(w1_e, C):
    """Pack one expert's W1 [D, F] into the phase-ordered [P, NF*ND*P]
    layout matching _w1_off."""
    ND, NF = D_MODEL // P, D_FF // P
    PHF = _phf(C)
    w1 = np.asarray(w1_e, dtype=np.float32).astype(_BF16)
    cur = w1.reshape(ND, P, NF, P)                 # [d, p, f, j]
    reg12 = cur[:, :, :PHF, :].transpose(1, 0, 2, 3).reshape(P, ND * PHF * P)
    reg3 = cur[:, :, PHF:, :].transpose(1, 2, 0, 3).reshape(
        P, (NF - PHF) * ND * P)
    return np.ascontiguousarray(np.concatenate([reg12, reg3], axis=1))


def kernel(x, anchors, temperature, W1, b1, W2, b2, top_k):

    x = np.asarray(x)
    B, S, D = x.shape
    T = B * S
    E = np.asarray(anchors).shape[0]
    k = int(np.asarray(top_k))

    xf = np.ascontiguousarray(x.reshape(T, D), dtype=np.float32)

    # ---- routing on host (part of the dispatch decision) ----
    xn = xf / np.maximum(np.linalg.norm(xf, axis=-1, keepdims=True), 1e-8)
    an = np.asarray(anchors, dtype=np.float32)
    an = an / np.maximum(np.linalg.norm(an, axis=-1, keepdims=True), 1e-8)
    scores = (xn @ an.T) * abs(float(np.asarray(temperature)))
    scores -= scores.max(axis=-1, keepdims=True)
    probs = np.exp(scores)
    probs /= probs.sum(axis=-1, keepdims=True)
    topi = np.argsort(-probs, axis=-1, kind="stable")[:, :k]  # ties -> low idx
    topv = np.take_along_axis(probs, topi, axis=-1)
    gw = topv / (topv.sum(axis=-1, keepdims=True) + 1e-6)

    rows_per_e = []
    gates_per_e = []
    for e in range(E):
        mask = topi == e
        rows = np.nonzero(mask.any(axis=-1))[0]
        g = np.where(mask[rows], gw[rows], 0.0).sum(axis=-1).astype(np.float32)
        rows_per_e.append(rows)
        gates_per_e.append(g)

    max_count = max(len(r) for r in rows_per_e)
    C = max(64, -(-max_count // 32) * 32)
    nc = _get_kernel(C)

    # ---- per-core shards, pre-arranged into SBUF layouts ----
    x_bf = xf.astype(_BF16)
    ND, NF = D_MODEL // P, D_FF // P
    in_maps = []
    for e in range(N_CORES):
        rows = rows_per_e[e]
        n = len(rows)
        xT = np.zeros((P, ND * C), dtype=_BF16)
        # [P, ND, C] view: xT[p, d, t] = x[rows[t], d*P + p]
        xv = xT.reshape(P, ND, C)
        xv[:, :, :n] = x_bf[rows].reshape(n, ND, P).transpose(2, 1, 0)
        w1 = _pack_w1(W1[e], C)
        w2 = np.asarray(W2[e], dtype=np.float32).astype(_BF16)
        w2 = np.ascontiguousarray(
            w2.reshape(NF, P, D_MODEL).transpose(1, 0, 2).reshape(P, NF * D_MODEL))
        meta = np.zeros((P, NF + ND + C), dtype=np.float32)
        meta[:, :NF] = np.asarray(b1[e], dtype=np.float32).reshape(NF, P).T
        meta[:, NF : NF + ND] = np.asarray(b2[e], dtype=np.float32).reshape(ND, P).T
        meta[:, NF + ND : NF + ND + n] = gates_per_e[e][None, :]
        in_maps.append({"xT": xT, "w1": w1, "w2": w2, "meta": meta})

    res = _run_spmd(nc, in_maps)
    global last_results
    last_results = res

    # ---- combine (scatter-add the gated expert outputs; device output
    # is gate-weighted W2@h, so add the gate-weighted b2 here) ----
    out = np.zeros((T, D_MODEL), dtype=np.float32)
    for e in range(N_CORES):
        rows = rows_per_e[e]
        n = len(rows)
        if n:
            out[rows] += res.results[e]["out"][:, :n].T.astype(np.float32)
            b2e = np.asarray(b2[e], dtype=np.float32)
            if b2e.any():
                out[rows] += gates_per_e[e][:, None] * b2e[None, :]
    return out.reshape(B, S, D_MODEL)
